# revision 16
# baseline (speedup 1.0000x reference)
"""MoE FFN (top-2 routing, 8 experts) on 8 Trainium2 NeuronCores.

Strategy (expert parallelism, per the sharding hint):
  - Host computes router logits / top-2 / softmax (tiny: T x E) and
    dispatches tokens: expert e's tokens are gathered into a padded
    [H, C] batch for core e (C = common capacity).
  - Core e runs the dense FFN for its expert on its gathered tokens:
        yT = ( GELU_tanh(x @ W1 + b1) @ W2 + b2 )^T
    computed fully transposed ([F,c] then [H,c]) so both matmuls use
    the weights as the stationary operand and no on-device transposes
    are needed. Matmul operands are fp16 (same PE rate as bf16 but 8x
    finer mantissa); accumulation is fp32 in PSUM.
  - The per-token combine weight is applied on the HOST during the
    scatter-add (cheap, and skips a whole [H, C] vector pass + the
    wtb upload on device).

Performance notes (v3, vs the 165 us baseline):
  - C is padded to a multiple of 4 (not 128) and split into equal
    chunks <= 512 wide: the PE streams 288*C rows instead of
    288*pad128(C) (C: 1152 -> 1068 for this input, ~10 us).
  - All DRAM operands are pre-packed on the host into the exact
    per-partition SBUF image ([128, *] with the tile's free-dim layout)
    so every DMA is a plain contiguous 2D copy: 128 descriptors with
    multi-KB lines instead of 768 strided 512B rows. Descriptor
    generation was the startup bottleneck: with strided pieces the two
    HWDGE rings only sustained ~270 GB/s and the PE starved until
    ~27 us; contiguous pieces stream at full rate.
  - DMA is enqueued in consumption order (small first W1 piece + first
    x chunk gate the first matmul at ~11 us instead of ~16.6 us).
  - Phase B of chunk 0 iterates fk-outer across 6 PSUM banks so W2
    streaming is spread over the whole phase; later chunks go hn-outer
    so each output row-tile's epilogue+store overlaps the remaining
    matmuls (short kernel tail).
  - PE warmup (~4 us of dummy matmuls, > the ~3.5 us HAM ramp
    threshold) lifts the clock 1.2 -> 2.4 GHz during the DMA-bound
    startup; the clock then stays up.

Self-contained: hardcodes the problem shapes (H=768, F=3072, E=8, K=2).
"""

import os
import time

import numpy as np

H = 768
F = 3072
E = 8
K = 2
N_CORES = 8
P = 128
FM = F // P   # 24 f row-tiles
HK = H // P   # 6 contraction tiles for x@W1
HN = H // P   # 6 output row-tiles of yT
W2G = 4       # fk tiles per W2 DMA piece
N_W2P = FM // W2G

PRECISION = os.environ.get("MOE_PRECISION", "fp16")  # "fp16" | "bf16" | "fp32"
WARMUP_MM = int(os.environ.get("MOE_WARMUP_MM", "34"))

# W1 column pieces (fm-consumption order): a small head piece so the
# first matmul can start early, then 256-col pieces.
W1_PIECES = [(0, P)] + [(P + 256 * i, 256) for i in range(11)] + [(F - P, P)]


def _w1_piece_of(fm):
    if fm == 0:
        return 0, 0
    return (fm + 1) // 2, (0 if fm % 2 == 1 else P)


def _chunks(C):
    """Split C columns into chunks of width <= 512 (PSUM bank limit).

    The first chunk is biased slightly wider: it is consumed while W1 is
    still streaming in, and a wider chunk consumes W1 pieces more slowly.
    """
    n = max(1, -(-C // 512))
    if n == 1:
        return [(0, C)]
    w0 = min(512, ((-(-C // n) + 28) // 2) * 2)
    rest = C - w0
    m = n - 1
    base, rem = divmod(rest, m)
    ws = [w0] + [base + 1] * rem + [base] * (m - rem)
    out, c0 = [], 0
    for w in ws:
        out.append((c0, w))
        c0 += w
    return out


# ---------------------------------------------------------------------------
# Bass/Tile device kernel
# ---------------------------------------------------------------------------

def _build_bass(C, precision=None):
    from contextlib import ExitStack

    import concourse.bass as bass  # noqa: F401
    import concourse.tile as tile
    from concourse import bacc, mybir
    from concourse._compat import with_exitstack

    precision = precision or PRECISION
    f32 = mybir.dt.float32
    mdt = {"bf16": mybir.dt.bfloat16, "fp16": mybir.dt.float16,
           "fp32": f32}[precision]

    chunks = _chunks(C)
    WMAX = max(w for _, w in chunks)

    nc = bacc.Bacc("TRN2", target_bir_lowering=False, debug=False,
                   num_devices=N_CORES)
    # All inputs are host-packed per-partition images (see _make_in_maps):
    # a DMA is always dst_tile[:] <- img[:, a:b] with contiguous rows.
    xgt = nc.dram_tensor("xgt", [P, HK * C], mdt, kind="ExternalInput").ap()
    w1 = nc.dram_tensor("w1", [P, HK * F], mdt, kind="ExternalInput").ap()
    w2 = nc.dram_tensor("w2", [P, FM * H], mdt, kind="ExternalInput").ap()
    cpk = nc.dram_tensor("cpk", [P, FM + HN], f32,
                         kind="ExternalInput").ap()
    y = nc.dram_tensor("y", [P, HN * C], mdt, kind="ExternalOutput").ap()

    gelu = mybir.ActivationFunctionType.Gelu_apprx_tanh
    ident = mybir.ActivationFunctionType.Identity

    @with_exitstack
    def body(ctx: ExitStack, tc: tile.TileContext):
        const = ctx.enter_context(tc.tile_pool(name="const", bufs=1))
        w1pool = ctx.enter_context(tc.tile_pool(name="w1pool", bufs=1))
        w2pool = ctx.enter_context(tc.tile_pool(name="w2pool", bufs=1))
        xp = ctx.enter_context(tc.tile_pool(name="xp", bufs=1))
        hp = ctx.enter_context(tc.tile_pool(name="hp", bufs=1))
        yp = ctx.enter_context(tc.tile_pool(name="yp", bufs=3))
        psAp = ctx.enter_context(tc.tile_pool(name="psA", bufs=2, space="PSUM"))
        psBp = ctx.enter_context(tc.tile_pool(name="psB", bufs=1, space="PSUM"))

        # --- PE warmup: ramp the HAM clock gate 1.2 -> 2.4 GHz during the
        # DMA-bound startup (needs >~3.5us of continuous PE busy; the clock
        # stays up once ramped).
        # A minimal 128-col memset (~120ns) lets the warmup start right
        # at PE preamble end; narrower warmup matmuls keep the ramp going.
        wtile = xp.tile([P, P], mdt, tag="warm", name="warm")
        nc.vector.memset(wtile[:], 0.0)
        wps = psBp.tile([P, WMAX], f32, tag="psB0", name="warmps")
        for i in range(WARMUP_MM):
            nc.tensor.matmul(wps[:, :P], lhsT=wtile[:],
                             rhs=wtile[:],
                             start=(i == 0), stop=(i == WARMUP_MM - 1))

        # --- SBUF tiles
        b12 = const.tile([P, FM + HN], f32, name="b12")
        b1s = b12[:, 0:FM]
        b2s = b12[:, FM:]
        w1t = [w1pool.tile([P, HK, wdt], mdt, tag=f"w1p{i}", name=f"w1p{i}")
               for i, (_, wdt) in enumerate(W1_PIECES)]
        w2t = [w2pool.tile([P, W2G, H], mdt, tag=f"w2p{g}", name=f"w2p{g}")
               for g in range(N_W2P)]
        xgs = [xp.tile([P, HK, w], mdt, tag=f"xg{ci}", name=f"xg{ci}")
               for ci, (_, w) in enumerate(chunks)]

        # --- DMA staging. Two rings (sync hwdge + gpsimd) share the HBM
        # pipe; jobs are enqueued in consumption order per ring. The
        # SCALAR engine must stay DMA-free: enqueueing descriptors blocks
        # it (ring-slot recycle waits pace enqueue at transfer speed), and
        # it has to run the phase-A activations from ~t0 on — otherwise
        # the PE stalls on PSUM recycling after two fm groups.
        def w1_dma(ring, i):
            c0, wdt = W1_PIECES[i]
            ring.dma_start(w1t[i][:], w1[:, HK * c0:HK * (c0 + wdt)])

        c00, w00 = chunks[0]
        nc.sync.dma_start(xgs[0][:], xgt[:, 0:HK * w00])
        w1_dma(nc.gpsimd, 0)
        nc.gpsimd.dma_start(b12[:], cpk[:])
        w1_dma(nc.sync, 1)
        for i in range(2, len(W1_PIECES)):
            w1_dma(nc.gpsimd if i % 2 == 0 else nc.sync, i)
        for ci in range(1, len(chunks)):
            c0, w = chunks[ci]
            (nc.sync if ci % 2 == 1 else nc.gpsimd).dma_start(
                xgs[ci][:], xgt[:, HK * c0:HK * (c0 + w)])
        for g in range(N_W2P):
            (nc.gpsimd if g % 2 == 0 else nc.sync).dma_start(
                w2t[g][:], w2[:, g * W2G * H:(g + 1) * W2G * H])

        def w1_tile(hk, fm):
            i, off = _w1_piece_of(fm)
            return w1t[i][:, hk, off:off + P]

        def w2_tile(fk, hn):
            return w2t[fk // W2G][:, fk % W2G, hn * P:(hn + 1) * P]

        psAs = [psAp.tile([P, WMAX], f32, tag="psA", name=f"psA{j}")
                for j in range(2)]
        psBs = [psBp.tile([P, WMAX], f32, tag=f"psB{j}", name=f"psB{j}")
                for j in range(HN)]
        htst = hp.tile([P, FM, WMAX], mdt, tag="hts", name="hts")
        yos = [yp.tile([P, HN, WMAX], mdt, tag="yout", name=f"yout{j}")
               for j in range(3)]

        pending_epi = []
        for ci, (c0, w) in enumerate(chunks):
            last = ci == len(chunks) - 1
            # ---- phase A: hT[f, c] = gelu((x@W1)[c, f] + b1[f]) ----
            hts = htst
            for fm in range(FM):
                ps = psAs[fm % 2]
                for hk in range(HK):
                    nc.tensor.matmul(
                        ps[:, :w],
                        lhsT=w1_tile(hk, fm),
                        rhs=xgs[ci][:, hk, :w],
                        start=(hk == 0), stop=(hk == HK - 1),
                    )
                nc.scalar.activation(hts[:, fm, :w], ps[:, :w], gelu,
                                     bias=b1s[:, fm:fm + 1])
                if fm == 1 and pending_epi:
                    # previous chunk's deferred phase-B epilogue: emitted
                    # after this chunk's first two A-activations so the
                    # scalar engine frees psA slots without a PE stall.
                    for f in pending_epi:
                        f()
                    pending_epi = []

            # ---- phase B: yT[h, c] = sum_f W2[f, h] * hT[f, c] (+b2) ----
            yo = yos[ci % 3]
            if ci == 0 and not last:
                # fk-outer across 6 PSUM banks: W2[fk] is consumed
                # progressively, so its DMA can stream during the phase.
                for fk in range(FM):
                    for hn in range(HN):
                        nc.tensor.matmul(
                            psBs[hn][:, :w],
                            lhsT=w2_tile(fk, hn),
                            rhs=hts[:, fk, :w],
                            start=(fk == 0), stop=(fk == FM - 1),
                        )
                def _epi(yo=yo, c0=c0, w=w):
                    for hn in range(HN):
                        nc.scalar.activation(yo[:, hn, :w], psBs[hn][:, :w],
                                             ident, bias=b2s[:, hn:hn + 1])
                    nc.sync.dma_start(y[:, HN * c0:HN * (c0 + w)],
                                      yo[:, :, :w])
                pending_epi.append(_epi)
            else:
                # hn-outer: epilogue + store of each row-tile overlap the
                # remaining matmuls (short kernel tail on the last chunk).
                for hn in range(HN):
                    ps = psBs[hn]
                    # On the very last output group, compute/store in two
                    # half-width column sub-groups so the final activation
                    # and store are half-size and overlap the first half's
                    # matmuls (shorter kernel tail).
                    if last and hn == HN - 1:
                        halves = ((0, w // 2), (w // 2, w))
                    else:
                        halves = ((0, w),)
                    for a, b in halves:
                        for fk in range(FM):
                            nc.tensor.matmul(
                                ps[:, a:b],
                                lhsT=w2_tile(fk, hn),
                                rhs=hts[:, fk, a:b],
                                start=(fk == 0), stop=(fk == FM - 1),
                            )
                        nc.scalar.activation(yo[:, hn, a:b], ps[:, a:b],
                                             ident, bias=b2s[:, hn:hn + 1])
                        if last:
                            nc.sync.dma_start(
                                y[:, HN * c0 + hn * w + a:
                                  HN * c0 + hn * w + b],
                                yo[:, hn, a:b])
                if not last:
                    nc.sync.dma_start(y[:, HN * c0:HN * (c0 + w)],
                                      yo[:, :, :w])

    with tile.TileContext(nc) as tc:
        body(tc)
    nc.compile()
    return nc


# ---------------------------------------------------------------------------
# Host-side routing + dispatch
# ---------------------------------------------------------------------------

def _route(xf, gate_w):
    """Top-2 router in float64 for a numerically robust top-k set."""
    logits = xf.astype(np.float64) @ gate_w.astype(np.float64)  # [T, E]
    top_idx = np.argpartition(logits, E - K, axis=1)[:, E - K:]  # [T, K]
    top_val = np.take_along_axis(logits, top_idx, axis=1)
    m = top_val.max(axis=1, keepdims=True)
    ex = np.exp(top_val - m)
    wts = ex / ex.sum(axis=1, keepdims=True)  # [T, K] float64

    toks, ws = [], []
    for e in range(E):
        mask = top_idx == e  # [T, K]
        rows = np.nonzero(mask.any(axis=1))[0]
        toks.append(rows)
        ws.append(wts[mask].astype(np.float32))
    return toks, ws


def _np_mdt():
    import ml_dtypes
    return {"bf16": ml_dtypes.bfloat16, "fp16": np.float16,
            "fp32": np.float32}[PRECISION]


def _pack_w1(W1e, mdt):
    """[H, F] -> [P, HK*F] image matching the w1 piece tiles."""
    w = np.asarray(W1e, np.float32).astype(mdt).reshape(HK, P, F)
    cols = [w[:, :, c0:c0 + wd].transpose(1, 0, 2).reshape(P, HK * wd)
            for c0, wd in W1_PIECES]
    return np.ascontiguousarray(np.concatenate(cols, axis=1))


def _pack_w2(W2e, mdt):
    """[F, H] -> [P, FM*H] image (fk-major blocks)."""
    w = np.asarray(W2e, np.float32).astype(mdt).reshape(FM, P, H)
    return np.ascontiguousarray(w.transpose(1, 0, 2).reshape(P, FM * H))


def _pack_xg(xT, chunks, mdt):
    """[H, C] -> [P, HK*C] image (chunk-major, hk-major within chunk)."""
    xr = xT.reshape(HK, P, -1)
    cols = [xr[:, :, c0:c0 + w].transpose(1, 0, 2).reshape(P, HK * w)
            for c0, w in chunks]
    return np.ascontiguousarray(np.concatenate(cols, axis=1).astype(mdt))


def _unpack_y(img, chunks, C):
    """[P, HN*C] image -> [H, C]."""
    y = np.empty((H, C), np.float32)
    for c0, w in chunks:
        blk = np.asarray(img[:, HN * c0:HN * (c0 + w)], np.float32)
        y[:, c0:c0 + w] = blk.reshape(P, HN, w).transpose(1, 0, 2).reshape(H, w)
    return y


def _run(inputs, trace=False):
    global PRECISION
    from concourse.bass_utils import run_bass_kernel_spmd

    x, gate_w, W1, b1, W2, b2 = (inputs[k] for k in
                                 ("x", "gate_w", "W1", "b1", "W2", "b2"))
    x = np.asarray(x)
    Bb, S, Hd = x.shape
    assert Hd == H
    T = Bb * S
    xf = np.ascontiguousarray(x.reshape(T, Hd), dtype=np.float32)
    gate_w = np.asarray(gate_w, np.float32)

    # fp16 matmul operands need moderate dynamic range; fall back to
    # bf16 (full fp32 exponent range) if the data is far outside the
    # expected unit-scale regime.
    if PRECISION == "fp16":
        amax = max(float(np.abs(np.asarray(t)).max())
                   for t in (xf, W1, W2))
        if not np.isfinite(amax) or amax > 1e3:
            PRECISION = "bf16"
    mdt = _np_mdt()

    toks, ws = _route(xf, gate_w)
    nmax = max(len(t) for t in toks)
    C = max(P, ((nmax + 3) // 4) * 4)
    chunks = _chunks(C)

    b1a = np.asarray(b1, np.float32)
    b2a = np.asarray(b2, np.float32)
    in_maps = []
    for e in range(E):
        n_e = len(toks[e])
        xT = np.zeros((H, C), np.float32)
        xT[:, :n_e] = xf[toks[e]].T
        cpk = np.concatenate([b1a[e].reshape(FM, P).T,
                              b2a[e].reshape(HN, P).T], axis=1)
        in_maps.append({
            "xgt": _pack_xg(xT, chunks, mdt),
            "w1": _pack_w1(W1[e], mdt),
            "w2": _pack_w2(W2[e], mdt),
            "cpk": np.ascontiguousarray(cpk),
        })

    nc = _build_bass(C)

    kwargs = {}
    if trace:
        kwargs = dict(trace=True, trace_cores=list(range(N_CORES)))
    try:
        res = run_bass_kernel_spmd(nc, in_maps, core_ids=list(range(N_CORES)),
                                   **kwargs)
    except Exception:
        # One retry for transient device faults.
        time.sleep(5)
        res = run_bass_kernel_spmd(nc, in_maps, core_ids=list(range(N_CORES)),
                                   **kwargs)
    out = np.zeros((T, H), np.float32)
    for e in range(E):
        n_e = len(toks[e])
        ye = _unpack_y(res.results[e]["y"], chunks, C)  # [H, C] fp32
        out[toks[e]] += ws[e][:, None] * ye[:, :n_e].T
    return out.reshape(Bb, S, Hd), res


def kernel(x, gate_w, W1, b1, W2, b2):
    out, _ = _run({"x": x, "gate_w": gate_w, "W1": W1, "b1": b1,
                   "W2": W2, "b2": b2})
    return out.astype(np.asarray(x).dtype, copy=False)


# Exposed for test.py: run with profiling, return (output, BassKernelResults)
def kernel_profiled(x, gate_w, W1, b1, W2, b2):
    return _run({"x": x, "gate_w": gate_w, "W1": W1, "b1": b1,
                 "W2": W2, "b2": b2}, trace=True)


# revision 17
# speedup vs baseline: 1.1970x; 1.1970x over previous
"""MoE FFN (top-2 routing, 8 experts) on 8 Trainium2 NeuronCores.

Strategy (expert parallelism, per the sharding hint):
  - Host computes router logits / top-2 / softmax (tiny: T x E) and
    dispatches tokens: expert e's tokens are gathered into a padded
    [H, C] batch for core e (C = common capacity).
  - Core e runs the dense FFN for its expert on its gathered tokens:
        yT = ( GELU_tanh(x @ W1 + b1) @ W2 + b2 )^T
    computed fully transposed ([F,c] then [H,c]) so both matmuls use
    the weights as the stationary operand and no on-device transposes
    are needed. Matmul operands are fp16 (same PE rate as bf16 but 8x
    finer mantissa); accumulation is fp32 in PSUM.
  - The per-token combine weight is applied on the HOST during the
    scatter-add (cheap, and skips a whole [H, C] vector pass + the
    wtb upload on device).

Performance notes (v3, vs the 165 us baseline):
  - C is padded to a multiple of 4 (not 128) and split into equal
    chunks <= 512 wide: the PE streams 288*C rows instead of
    288*pad128(C) (C: 1152 -> 1068 for this input, ~10 us).
  - All DRAM operands are pre-packed on the host into the exact
    per-partition SBUF image ([128, *] with the tile's free-dim layout)
    so every DMA is a plain contiguous 2D copy: 128 descriptors with
    multi-KB lines instead of 768 strided 512B rows. Descriptor
    generation was the startup bottleneck: with strided pieces the two
    HWDGE rings only sustained ~270 GB/s and the PE starved until
    ~27 us; contiguous pieces stream at full rate.
  - DMA is enqueued in consumption order (small first W1 piece + first
    x chunk gate the first matmul at ~11 us instead of ~16.6 us).
  - Phase B of chunk 0 iterates fk-outer across 6 PSUM banks so W2
    streaming is spread over the whole phase; later chunks go hn-outer
    so each output row-tile's epilogue+store overlaps the remaining
    matmuls (short kernel tail).
  - PE warmup (~4 us of dummy matmuls, > the ~3.5 us HAM ramp
    threshold) lifts the clock 1.2 -> 2.4 GHz during the DMA-bound
    startup; the clock then stays up.

Self-contained: hardcodes the problem shapes (H=768, F=3072, E=8, K=2).
"""

import os
import time

import numpy as np

H = 768
F = 3072
E = 8
K = 2
N_CORES = 8
P = 128
FM = F // P   # 24 f row-tiles
HK = H // P   # 6 contraction tiles for x@W1
HN = H // P   # 6 output row-tiles of yT
W2G = 4       # fk tiles per W2 DMA piece
N_W2P = FM // W2G

PRECISION = os.environ.get("MOE_PRECISION", "fp16")  # "fp16" | "bf16" | "fp32"
WARMUP_MM = int(os.environ.get("MOE_WARMUP_MM", "14"))

# W1 column pieces (fm-consumption order): a small head piece so the
# first matmul can start early, then 256-col pieces.
W1_PIECES = [(0, P)] + [(P + 256 * i, 256) for i in range(11)] + [(F - P, P)]


def _w1_piece_of(fm):
    if fm == 0:
        return 0, 0
    return (fm + 1) // 2, (0 if fm % 2 == 1 else P)


def _chunks(C):
    """Split C columns into chunks of width <= 512 (PSUM bank limit).

    The first chunk is biased slightly wider: it is consumed while W1 is
    still streaming in, and a wider chunk consumes W1 pieces more slowly.
    """
    n = max(1, -(-C // 512))
    if n == 1:
        return [(0, C)]
    w0 = min(512, ((-(-C // n) + 28) // 2) * 2)
    rest = C - w0
    m = n - 1
    base, rem = divmod(rest, m)
    ws = [w0] + [base + 1] * rem + [base] * (m - rem)
    out, c0 = [], 0
    for w in ws:
        out.append((c0, w))
        c0 += w
    return out


# ---------------------------------------------------------------------------
# Bass/Tile device kernel
# ---------------------------------------------------------------------------

def _build_bass(C, precision=None):
    from contextlib import ExitStack

    import concourse.bass as bass  # noqa: F401
    import concourse.tile as tile
    from concourse import bacc, mybir
    from concourse._compat import with_exitstack

    precision = precision or PRECISION
    f32 = mybir.dt.float32
    mdt = {"bf16": mybir.dt.bfloat16, "fp16": mybir.dt.float16,
           "fp32": f32}[precision]

    chunks = _chunks(C)
    WMAX = max(w for _, w in chunks)

    nc = bacc.Bacc("TRN2", target_bir_lowering=False, debug=False,
                   num_devices=N_CORES)
    # All inputs are host-packed per-partition images (see _make_in_maps):
    # a DMA is always dst_tile[:] <- img[:, a:b] with contiguous rows.
    xgt = nc.dram_tensor("xgt", [P, HK * C], mdt, kind="ExternalInput").ap()
    w1 = nc.dram_tensor("w1", [P, HK * F], mdt, kind="ExternalInput").ap()
    w2 = nc.dram_tensor("w2", [P, FM * H], mdt, kind="ExternalInput").ap()
    cpk = nc.dram_tensor("cpk", [P, FM + HN], f32,
                         kind="ExternalInput").ap()
    y = nc.dram_tensor("y", [P, HN * C], mdt, kind="ExternalOutput").ap()

    gelu = mybir.ActivationFunctionType.Gelu_apprx_tanh
    ident = mybir.ActivationFunctionType.Identity

    @with_exitstack
    def body(ctx: ExitStack, tc: tile.TileContext):
        const = ctx.enter_context(tc.tile_pool(name="const", bufs=1))
        w1pool = ctx.enter_context(tc.tile_pool(name="w1pool", bufs=1))
        w2pool = ctx.enter_context(tc.tile_pool(name="w2pool", bufs=1))
        xp = ctx.enter_context(tc.tile_pool(name="xp", bufs=1))
        hp = ctx.enter_context(tc.tile_pool(name="hp", bufs=1))
        yp = ctx.enter_context(tc.tile_pool(name="yp", bufs=3))
        psAp = ctx.enter_context(tc.tile_pool(name="psA", bufs=2, space="PSUM"))
        psBp = ctx.enter_context(tc.tile_pool(name="psB", bufs=1, space="PSUM"))

        # --- PE warmup: ramp the HAM clock gate 1.2 -> 2.4 GHz during the
        # DMA-bound startup (needs >~3.5us of continuous PE busy; the clock
        # stays up once ramped).
        wtile = xp.tile([P, 512], mdt, tag="warm", name="warm")
        nc.vector.memset(wtile[:], 0.0)
        wps = psBp.tile([P, WMAX], f32, tag="psB0", name="warmps")
        for i in range(WARMUP_MM):
            nc.tensor.matmul(wps[:, :WMAX], lhsT=wtile[:, 0:P],
                             rhs=wtile[:, 0:WMAX],
                             start=(i == 0), stop=(i == WARMUP_MM - 1))

        # --- SBUF tiles
        b12 = const.tile([P, FM + HN], f32, name="b12")
        b1s = b12[:, 0:FM]
        b2s = b12[:, FM:]
        w1t = [w1pool.tile([P, HK, wdt], mdt, tag=f"w1p{i}", name=f"w1p{i}")
               for i, (_, wdt) in enumerate(W1_PIECES)]
        w2t = [w2pool.tile([P, W2G, H], mdt, tag=f"w2p{g}", name=f"w2p{g}")
               for g in range(N_W2P)]
        xgs = [xp.tile([P, HK, w], mdt, tag=f"xg{ci}", name=f"xg{ci}")
               for ci, (_, w) in enumerate(chunks)]

        # --- DMA staging. Two rings (sync hwdge + gpsimd) share the HBM
        # pipe; jobs are enqueued in consumption order per ring. The
        # SCALAR engine must stay DMA-free: enqueueing descriptors blocks
        # it (ring-slot recycle waits pace enqueue at transfer speed), and
        # it has to run the phase-A activations from ~t0 on — otherwise
        # the PE stalls on PSUM recycling after two fm groups.
        def w1_dma(ring, i):
            c0, wdt = W1_PIECES[i]
            ring.dma_start(w1t[i][:], w1[:, HK * c0:HK * (c0 + wdt)])

        c00, w00 = chunks[0]
        nc.sync.dma_start(xgs[0][:], xgt[:, 0:HK * w00])
        w1_dma(nc.gpsimd, 0)
        nc.gpsimd.dma_start(b12[:], cpk[:])
        w1_dma(nc.sync, 1)
        for i in range(2, len(W1_PIECES)):
            w1_dma(nc.gpsimd if i % 2 == 0 else nc.sync, i)
        for ci in range(1, len(chunks)):
            c0, w = chunks[ci]
            (nc.sync if ci % 2 == 1 else nc.gpsimd).dma_start(
                xgs[ci][:], xgt[:, HK * c0:HK * (c0 + w)])
        for g in range(N_W2P):
            (nc.gpsimd if g % 2 == 0 else nc.sync).dma_start(
                w2t[g][:], w2[:, g * W2G * H:(g + 1) * W2G * H])

        def w1_tile(hk, fm):
            i, off = _w1_piece_of(fm)
            return w1t[i][:, hk, off:off + P]

        def w2_tile(fk, hn):
            return w2t[fk // W2G][:, fk % W2G, hn * P:(hn + 1) * P]

        psAs = [psAp.tile([P, WMAX], f32, tag="psA", name=f"psA{j}")
                for j in range(2)]
        psBs = [psBp.tile([P, WMAX], f32, tag=f"psB{j}", name=f"psB{j}")
                for j in range(HN)]
        htst = hp.tile([P, FM, WMAX], mdt, tag="hts", name="hts")
        yos = [yp.tile([P, HN, WMAX], mdt, tag="yout", name=f"yout{j}")
               for j in range(3)]

        pending_epi = []
        for ci, (c0, w) in enumerate(chunks):
            last = ci == len(chunks) - 1
            # ---- phase A: hT[f, c] = gelu((x@W1)[c, f] + b1[f]) ----
            hts = htst
            for fm in range(FM):
                ps = psAs[fm % 2]
                for hk in range(HK):
                    nc.tensor.matmul(
                        ps[:, :w],
                        lhsT=w1_tile(hk, fm),
                        rhs=xgs[ci][:, hk, :w],
                        start=(hk == 0), stop=(hk == HK - 1),
                    )
                nc.scalar.activation(hts[:, fm, :w], ps[:, :w], gelu,
                                     bias=b1s[:, fm:fm + 1])
                if fm == 1 and pending_epi:
                    # previous chunk's deferred phase-B epilogue: emitted
                    # after this chunk's first two A-activations so the
                    # scalar engine frees psA slots without a PE stall.
                    for f in pending_epi:
                        f()
                    pending_epi = []

            # ---- phase B: yT[h, c] = sum_f W2[f, h] * hT[f, c] (+b2) ----
            yo = yos[ci % 3]
            if ci == 0 and not last:
                # fk-outer across 6 PSUM banks: W2[fk] is consumed
                # progressively, so its DMA can stream during the phase.
                for fk in range(FM):
                    for hn in range(HN):
                        nc.tensor.matmul(
                            psBs[hn][:, :w],
                            lhsT=w2_tile(fk, hn),
                            rhs=hts[:, fk, :w],
                            start=(fk == 0), stop=(fk == FM - 1),
                        )
                def _epi(yo=yo, c0=c0, w=w):
                    for hn in range(HN):
                        nc.scalar.activation(yo[:, hn, :w], psBs[hn][:, :w],
                                             ident, bias=b2s[:, hn:hn + 1])
                    nc.sync.dma_start(y[:, HN * c0:HN * (c0 + w)],
                                      yo[:, :, :w])
                pending_epi.append(_epi)
            else:
                # hn-outer: epilogue + store of each row-tile overlap the
                # remaining matmuls (short kernel tail on the last chunk).
                for hn in range(HN):
                    ps = psBs[hn]
                    # On the very last output group, compute/store in two
                    # half-width column sub-groups so the final activation
                    # and store are half-size and overlap the first half's
                    # matmuls (shorter kernel tail).
                    if last and hn == HN - 1:
                        halves = ((0, w // 2), (w // 2, w))
                    else:
                        halves = ((0, w),)
                    for a, b in halves:
                        for fk in range(FM):
                            nc.tensor.matmul(
                                ps[:, a:b],
                                lhsT=w2_tile(fk, hn),
                                rhs=hts[:, fk, a:b],
                                start=(fk == 0), stop=(fk == FM - 1),
                            )
                        nc.scalar.activation(yo[:, hn, a:b], ps[:, a:b],
                                             ident, bias=b2s[:, hn:hn + 1])
                        if last:
                            nc.sync.dma_start(
                                y[:, HN * c0 + hn * w + a:
                                  HN * c0 + hn * w + b],
                                yo[:, hn, a:b])
                if not last:
                    nc.sync.dma_start(y[:, HN * c0:HN * (c0 + w)],
                                      yo[:, :, :w])

    with tile.TileContext(nc) as tc:
        body(tc)
    nc.compile()
    return nc


# ---------------------------------------------------------------------------
# Host-side routing + dispatch
# ---------------------------------------------------------------------------

def _route(xf, gate_w):
    """Top-2 router in float64 for a numerically robust top-k set."""
    logits = xf.astype(np.float64) @ gate_w.astype(np.float64)  # [T, E]
    top_idx = np.argpartition(logits, E - K, axis=1)[:, E - K:]  # [T, K]
    top_val = np.take_along_axis(logits, top_idx, axis=1)
    m = top_val.max(axis=1, keepdims=True)
    ex = np.exp(top_val - m)
    wts = ex / ex.sum(axis=1, keepdims=True)  # [T, K] float64

    toks, ws = [], []
    for e in range(E):
        mask = top_idx == e  # [T, K]
        rows = np.nonzero(mask.any(axis=1))[0]
        toks.append(rows)
        ws.append(wts[mask].astype(np.float32))
    return toks, ws


def _np_mdt():
    import ml_dtypes
    return {"bf16": ml_dtypes.bfloat16, "fp16": np.float16,
            "fp32": np.float32}[PRECISION]


def _pack_w1(W1e, mdt):
    """[H, F] -> [P, HK*F] image matching the w1 piece tiles."""
    w = np.asarray(W1e, np.float32).astype(mdt).reshape(HK, P, F)
    cols = [w[:, :, c0:c0 + wd].transpose(1, 0, 2).reshape(P, HK * wd)
            for c0, wd in W1_PIECES]
    return np.ascontiguousarray(np.concatenate(cols, axis=1))


def _pack_w2(W2e, mdt):
    """[F, H] -> [P, FM*H] image (fk-major blocks)."""
    w = np.asarray(W2e, np.float32).astype(mdt).reshape(FM, P, H)
    return np.ascontiguousarray(w.transpose(1, 0, 2).reshape(P, FM * H))


def _pack_xg(xT, chunks, mdt):
    """[H, C] -> [P, HK*C] image (chunk-major, hk-major within chunk)."""
    xr = xT.reshape(HK, P, -1)
    cols = [xr[:, :, c0:c0 + w].transpose(1, 0, 2).reshape(P, HK * w)
            for c0, w in chunks]
    return np.ascontiguousarray(np.concatenate(cols, axis=1).astype(mdt))


def _unpack_y(img, chunks, C):
    """[P, HN*C] image -> [H, C]."""
    y = np.empty((H, C), np.float32)
    for c0, w in chunks:
        blk = np.asarray(img[:, HN * c0:HN * (c0 + w)], np.float32)
        y[:, c0:c0 + w] = blk.reshape(P, HN, w).transpose(1, 0, 2).reshape(H, w)
    return y


def _run(inputs, trace=False):
    global PRECISION
    from concourse.bass_utils import run_bass_kernel_spmd

    x, gate_w, W1, b1, W2, b2 = (inputs[k] for k in
                                 ("x", "gate_w", "W1", "b1", "W2", "b2"))
    x = np.asarray(x)
    Bb, S, Hd = x.shape
    assert Hd == H
    T = Bb * S
    xf = np.ascontiguousarray(x.reshape(T, Hd), dtype=np.float32)
    gate_w = np.asarray(gate_w, np.float32)

    # fp16 matmul operands need moderate dynamic range; fall back to
    # bf16 (full fp32 exponent range) if the data is far outside the
    # expected unit-scale regime.
    if PRECISION == "fp16":
        amax = max(float(np.abs(np.asarray(t)).max())
                   for t in (xf, W1, W2))
        if not np.isfinite(amax) or amax > 1e3:
            PRECISION = "bf16"
    mdt = _np_mdt()

    toks, ws = _route(xf, gate_w)
    nmax = max(len(t) for t in toks)
    C = max(P, ((nmax + 3) // 4) * 4)
    chunks = _chunks(C)

    b1a = np.asarray(b1, np.float32)
    b2a = np.asarray(b2, np.float32)
    in_maps = []
    for e in range(E):
        n_e = len(toks[e])
        xT = np.zeros((H, C), np.float32)
        xT[:, :n_e] = xf[toks[e]].T
        cpk = np.concatenate([b1a[e].reshape(FM, P).T,
                              b2a[e].reshape(HN, P).T], axis=1)
        in_maps.append({
            "xgt": _pack_xg(xT, chunks, mdt),
            "w1": _pack_w1(W1[e], mdt),
            "w2": _pack_w2(W2[e], mdt),
            "cpk": np.ascontiguousarray(cpk),
        })

    nc = _build_bass(C)

    kwargs = {}
    if trace:
        kwargs = dict(trace=True, trace_cores=list(range(N_CORES)))
    try:
        res = run_bass_kernel_spmd(nc, in_maps, core_ids=list(range(N_CORES)),
                                   **kwargs)
    except Exception:
        # One retry for transient device faults.
        time.sleep(5)
        res = run_bass_kernel_spmd(nc, in_maps, core_ids=list(range(N_CORES)),
                                   **kwargs)
    out = np.zeros((T, H), np.float32)
    for e in range(E):
        n_e = len(toks[e])
        ye = _unpack_y(res.results[e]["y"], chunks, C)  # [H, C] fp32
        out[toks[e]] += ws[e][:, None] * ye[:, :n_e].T
    return out.reshape(Bb, S, Hd), res


def kernel(x, gate_w, W1, b1, W2, b2):
    out, _ = _run({"x": x, "gate_w": gate_w, "W1": W1, "b1": b1,
                   "W2": W2, "b2": b2})
    return out.astype(np.asarray(x).dtype, copy=False)


# Exposed for test.py: run with profiling, return (output, BassKernelResults)
def kernel_profiled(x, gate_w, W1, b1, W2, b2):
    return _run({"x": x, "gate_w": gate_w, "W1": W1, "b1": b1,
                 "W2": W2, "b2": b2}, trace=True)
